# revision 49
# baseline (speedup 1.0000x reference)
"""MinGRU layer (LN -> gate/candidate Linear -> minGRU scan -> residual) on 8 trn2 cores.

Problem (hardcoded): x [B=4, T=4096, H=1024] fp32, weights Wg/Wc [1024,1024],
biases bg/bc [1024], LN gamma/beta [1024].

Sharding: core c = (batch b = c//2, output-half p = c%2). Each core computes
z/c for its 512 output channels over the full sequence; the minGRU recurrence
is elementwise over (b, h) so output-channel sharding needs no collectives.

Strategy: all LayerNorm work (mu/var/rstd + normalize) happens on the HOST in
exact fp32; the device receives the already-normalized x quantized to fp8e4
plus fp8e4 weights pre-scaled by S=256 (so |S*W| stays in e4m3's normal
range). Device work per 512-token chunk is then just:
  - 32 DoubleRow fp8 matmuls (2 k-tiles of 128 per instruction, fp32 PSUM):
    gate and candidate GEMMs for 4 o-tiles.
  - z = sigmoid(pg/S + bg), a = 1-z = sigmoid(-pg/S - bg) on ScalarE
    (descale folded into the activation scale; only Sigmoid is ever used so
    the ACT table is loaded exactly once).
  - bsc = (pc + S*bc) * z on VectorE (stt); S stays folded into the scan.
  - h' = tensor_tensor_scan(a, bsc) on VectorE, chained across chunks
    (h' = S*h -- the scan is linear in its additive input); DMA h' out.
The residual (out = h'/S + x) is applied on the host in exact fp32.
"""

import functools
import os
import numpy as np
import ml_dtypes

import concourse.bacc as bacc
import concourse.hw_specs as hw_specs
import concourse.tile as tile
from concourse import mybir
from concourse.bass_utils import run_bass_kernel_spmd

# The table-load pass assigns each activation the FIRST act_func_set that
# contains it; the preamble's Copy/Identity land in exp_and_others while
# Sigmoid lives in sigmoid_and_others, costing a second ~1.3us ACT_TABLE_LOAD
# that blocks the first sigmoid. Strip sigmoid_and_others' functions from
# every other set so a single table serves the whole kernel.
_orig_get_act_tables = hw_specs.get_activation_tables


@functools.cache
def _patched_get_act_tables(module_arch):
    d = dict(_orig_get_act_tables(module_arch))
    keep = "sigmoid_and_others"
    if keep in d:
        for name in d:
            if name != keep:
                d[name] = d[name] - d[keep]
    return d


hw_specs.get_activation_tables = _patched_get_act_tables
bacc.get_activation_tables = _patched_get_act_tables

B, T, H = 4, 4096, 1024
EPS = 1e-5
N_CORES = 8
OH = H // 2          # output channels per core
CHUNK = 512
N_CHUNKS = T // CHUNK
KT = H // 128        # k-tiles (contraction)
KP = KT // 2         # DoubleRow k-pairs per accumulation group
OT = OH // 128       # o-tiles per core
S = 256.0            # weight pre-scale so fp8 weights use e4m3 normal range

F32 = mybir.dt.float32
BF16 = mybir.dt.bfloat16
FP8 = mybir.dt.float8e4
AF = mybir.ActivationFunctionType
OP = mybir.AluOpType
DR = mybir.MatmulPerfMode.DoubleRow
F8 = ml_dtypes.float8_e4m3

_CACHE = {}


def _build():
    nc = bacc.Bacc("TRN2", target_bir_lowering=False, debug=False)

    # host-pre-tiled so every DMA is fully contiguous
    xn_d = nc.dram_tensor("xn", [N_CHUNKS, 128, KT, CHUNK], FP8, kind="ExternalInput").ap()
    wg_d = nc.dram_tensor("wg", [128, KT, OH], FP8, kind="ExternalInput").ap()
    wc_d = nc.dram_tensor("wc", [128, KT, OH], FP8, kind="ExternalInput").ap()
    bg_d = nc.dram_tensor("bg", [128, OT], F32, kind="ExternalInput").ap()
    bgn_d = nc.dram_tensor("bgn", [128, OT], F32, kind="ExternalInput").ap()
    bcs_d = nc.dram_tensor("bcs", [128, OT], F32, kind="ExternalInput").ap()
    out_d = nc.dram_tensor("outT", [N_CHUNKS, OT, 128, CHUNK], F32, kind="ExternalOutput").ap()

    with tile.TileContext(nc) as tc:
        with (
            tc.tile_pool(name="const", bufs=1) as cpool,
            tc.tile_pool(name="xin", bufs=3) as xpool,
            tc.tile_pool(name="work", bufs=2) as wpool,
            tc.tile_pool(name="hbuf", bufs=2) as hpool,
            tc.tile_pool(name="psA", bufs=4, space="PSUM") as psA,
            tc.tile_pool(name="psB", bufs=4, space="PSUM") as psB,
        ):
            # resident constants; weights ride the scalar queue so the first
            # x chunk (sync queue) isn't stuck behind them
            wg_sb = cpool.tile([128, KT, OH], FP8, tag="wg")
            nc.scalar.dma_start(wg_sb[:, 0:4, :], wg_d[:, 0:4, :])
            nc.scalar.dma_start(wg_sb[:, 4:KT, :], wg_d[:, 4:KT, :])
            wc_sb = cpool.tile([128, KT, OH], FP8, tag="wc")
            nc.scalar.dma_start(wc_sb[:, 0:4, :], wc_d[:, 0:4, :])
            nc.scalar.dma_start(wc_sb[:, 4:KT, :], wc_d[:, 4:KT, :])
            # bias DMAs are issued on the sync queue after xn0 (below): they
            # are tiny and must not queue behind 1 MiB of weights, or the
            # first stt/sigmoid chain starts ~5us late
            bg_sb = cpool.tile([128, OT], F32, tag="bg")
            bgn_sb = cpool.tile([128, OT], F32, tag="bgn")
            bcs_sb = cpool.tile([128, OT], F32, tag="bcs")


            h_prev = [None] * OT     # (tile, last column index)
            xc_t = [None] * N_CHUNKS

            def load_x(i, split=False):
                xc = xpool.tile([128, KT, CHUNK], FP8, tag="xc")
                if split:  # first chunk: let o=0's first k-pairs start earlier
                    half = KT // 2
                    nc.sync.dma_start(xc[:, :half, :], xn_d[i, :, :half, :])
                    nc.sync.dma_start(xc[:, half:, :], xn_d[i, :, half:, :])
                else:
                    nc.sync.dma_start(xc[:], xn_d[i])
                xc_t[i] = xc

            # chunk pairs (0,1),(2,3),(4,5) share one scan per o-tile on
            # VectorE ([128,1024] halves the per-instruction overhead); the
            # per-chunk stt writes into the pair tile's halves. Chunks 6,7
            # run single so the pipeline tail stays short.
            # chunks 0,1 also run single: the first pair's scan would
            # otherwise wait for chunk 1's stt, idling VectorE ~2us at start
            PAIRED = {2, 3, 4, 5}
            ap_t = [None] * OT   # per-o pair tiles carried even->odd chunk
            bp_t = [None] * OT

            load_x(0, split=True)
            nc.sync.dma_start(bg_sb[:], bg_d[:])
            nc.sync.dma_start(bgn_sb[:], bgn_d[:])
            nc.sync.dma_start(bcs_sb[:], bcs_d[:])
            load_x(1)
            for i in range(N_CHUNKS):
                if i + 1 < N_CHUNKS and i > 0:
                    load_x(i + 1)
                xc = xc_t[i]
                paired = i in PAIRED
                half = i % 2 if paired else 0
                for o in range(OT):
                    og = o * 128
                    pg = psA.tile([128, CHUNK], F32, tag="pg")
                    for j in range(KP):
                        nc.tensor.matmul(
                            pg[:], wg_sb[:, 2 * j : 2 * j + 2, og : og + 128],
                            xc[:, 2 * j : 2 * j + 2, :],
                            start=(j == 0), stop=(j == KP - 1), perf_mode=DR,
                        )
                    pc = psB.tile([128, CHUNK], F32, tag="pc")
                    for j in range(KP):
                        nc.tensor.matmul(
                            pc[:], wc_sb[:, 2 * j : 2 * j + 2, og : og + 128],
                            xc[:, 2 * j : 2 * j + 2, :],
                            start=(j == 0), stop=(j == KP - 1), perf_mode=DR,
                        )

                    if paired and half == 0:
                        ap_t[o] = wpool.tile([128, 2 * CHUNK], F32, tag=f"a{o}", name="apair")
                        bp_t[o] = wpool.tile([128, 2 * CHUNK], F32, tag=f"b{o}", name="bpair")
                    if paired:
                        a = ap_t[o][:, half * CHUNK : (half + 1) * CHUNK]
                        bsc = bp_t[o][:, half * CHUNK : (half + 1) * CHUNK]
                    else:
                        a_t = wpool.tile([128, CHUNK], F32, tag=f"a{o}", name="a_t")
                        b_t = wpool.tile([128, CHUNK], F32, tag=f"b{o}", name="b_t")
                        a = a_t[:]
                        bsc = b_t[:]
                    z = wpool.tile([128, CHUNK], F32, tag="z", bufs=4)
                    nc.scalar.activation(
                        z[:], pg[:], AF.Sigmoid, bias=bg_sb[:, o : o + 1], scale=1.0 / S
                    )
                    if i == 0:
                        # chunk 0 only: emit the stt right after its real
                        # dependency (z) so its coarse cross-engine tick-wait
                        # doesn't include later sigmoids -- starts the
                        # saturated VectorE pipeline earlier. (Doing this for
                        # every chunk delays the a-sigmoids and starves the
                        # scans -- measured worse.)
                        nc.vector.scalar_tensor_tensor(
                            bsc, pc[:], bcs_sb[:, o : o + 1], z[:], OP.add, OP.mult
                        )
                        nc.scalar.activation(
                            a, pg[:], AF.Sigmoid, bias=bgn_sb[:, o : o + 1], scale=-1.0 / S
                        )
                    else:
                        nc.scalar.activation(
                            a, pg[:], AF.Sigmoid, bias=bgn_sb[:, o : o + 1], scale=-1.0 / S
                        )
                        nc.vector.scalar_tensor_tensor(
                            bsc, pc[:], bcs_sb[:, o : o + 1], z[:], OP.add, OP.mult
                        )

                    if paired and half == 0:
                        continue  # scan fires on the odd half over [128,1024]
                    if paired:
                        af, bf_, W = ap_t[o][:], bp_t[o][:], 2 * CHUNK
                    else:
                        af, bf_, W = a, bsc, CHUNK
                    h = hpool.tile([128, W], F32, tag=f"h{o}")
                    if h_prev[o] is None:
                        init = 0.0
                    else:
                        pt, pcol = h_prev[o]
                        init = pt[:, pcol : pcol + 1]
                    nc.vector.tensor_tensor_scan(h[:], af, bf_, init, OP.mult, OP.add)
                    h_prev[o] = (h, W - 1)
                    if paired:
                        nc.sync.dma_start(out_d[i - 1, o], h[:, 0:CHUNK])
                        nc.sync.dma_start(out_d[i, o], h[:, CHUNK : 2 * CHUNK])
                    else:
                        nc.sync.dma_start(out_d[i, o], h[:])

    nc.compile()
    return nc


def _prep_weights(gamma, beta, Wg, bg, Wc, bc, ohalf):
    """Host-side weight folding for one output half.

    h-rows of the weights (and of xn/xr) are rolled so this half's own output
    channels come first: the device residual then always reads x rows at
    k-tiles 0..OT-1 with one shared program across cores.
    """
    o0 = ohalf * OH
    perm = np.roll(np.arange(H), -o0)
    Wg_h = Wg[o0 : o0 + OH]          # [OH, H]
    Wc_h = Wc[o0 : o0 + OH]
    # lhsT layout [h, o], gamma folded into rows (h), rows permuted like xn
    wg_eff = ((Wg_h * gamma[None, :]).T)[perm]   # [H, OH]
    wc_eff = ((Wc_h * gamma[None, :]).T)[perm]
    bg_eff = (bg[o0 : o0 + OH] + Wg_h @ beta).astype(np.float32)
    bc_eff = (bc[o0 : o0 + OH] + Wc_h @ beta).astype(np.float32)

    def q8(w):  # [H, OH] -> fp8 tiles [128, KT, OH]
        w8 = np.clip(S * w, -240, 240).astype(F8)
        return np.ascontiguousarray(w8.reshape(KT, 128, OH).transpose(1, 0, 2))

    return {
        "wg": q8(wg_eff),
        "wc": q8(wc_eff),
        "bg": np.ascontiguousarray(bg_eff.reshape(OT, 128).T),
        "bgn": np.ascontiguousarray(-bg_eff.reshape(OT, 128).T),
        "bcs": np.ascontiguousarray(S * bc_eff.reshape(OT, 128).T),
    }


def kernel(x, gamma, beta, Wg, bg, Wc, bc):
    x = np.asarray(x, dtype=np.float32)
    gamma = np.asarray(gamma, dtype=np.float32)
    beta = np.asarray(beta, dtype=np.float32)
    Wg = np.asarray(Wg, dtype=np.float32)
    bg = np.asarray(bg, dtype=np.float32)
    Wc = np.asarray(Wc, dtype=np.float32)
    bc = np.asarray(bc, dtype=np.float32)

    if "nc" not in _CACHE:
        _CACHE["nc"] = _build()
    nc = _CACHE["nc"]

    # exact LN on host; gamma/beta fold into the weights/biases
    mu = x.mean(-1, keepdims=True)
    var = ((x - mu) ** 2).mean(-1, keepdims=True)
    normed = (x - mu) / np.sqrt(var + EPS)
    xn8 = np.clip(normed, -240, 240).astype(F8)

    halves = [_prep_weights(gamma, beta, Wg, bg, Wc, bc, p) for p in range(2)]

    in_maps = []
    for c in range(N_CORES):
        b, p = divmod(c, 2)
        m = dict(halves[p])
        # roll h-rows to match the weight-row permutation for this half
        xnT = xn8[b].T if p == 0 else np.roll(xn8[b].T, -OH, axis=0)   # [H, T]
        m["xn"] = np.ascontiguousarray(
            xnT.reshape(KT, 128, N_CHUNKS, CHUNK).transpose(2, 1, 0, 3)
        )
        in_maps.append(m)

    trace = bool(int(os.environ.get("MINGRU_TRACE", "0")))
    kwargs = {}
    if trace:
        tmpdir = os.environ.get("MINGRU_TRACE_DIR") or None
        kwargs = dict(trace=True, tmpdir=tmpdir)
    res = run_bass_kernel_spmd(nc, in_maps, core_ids=list(range(N_CORES)), **kwargs)
    if trace:
        _CACHE["last_results"] = res

    out = np.empty((B, T, H), dtype=np.float32)
    for c in range(N_CORES):
        b, p = divmod(c, 2)
        # [chunks, OT, 128, CHUNK] -> [OH, T] -> [T, OH]; h'/S + x residual
        oT = res.results[c]["outT"].transpose(1, 2, 0, 3).reshape(OH, T)
        sl = slice(p * OH, (p + 1) * OH)
        out[b, :, sl] = oT.T * (1.0 / S) + x[b, :, sl]
    return out


# revision 50
# speedup vs baseline: 1.0102x; 1.0102x over previous
"""MinGRU layer (LN -> gate/candidate Linear -> minGRU scan -> residual) on 8 trn2 cores.

Problem (hardcoded): x [B=4, T=4096, H=1024] fp32, weights Wg/Wc [1024,1024],
biases bg/bc [1024], LN gamma/beta [1024].

Sharding: core c = (batch b = c//2, output-half p = c%2). Each core computes
z/c for its 512 output channels over the full sequence; the minGRU recurrence
is elementwise over (b, h) so output-channel sharding needs no collectives.

Strategy: all LayerNorm work (mu/var/rstd + normalize) happens on the HOST in
exact fp32; the device receives the already-normalized x quantized to fp8e4
plus fp8e4 weights pre-scaled by S=256 (so |S*W| stays in e4m3's normal
range). Device work per 512-token chunk is then just:
  - 32 DoubleRow fp8 matmuls (2 k-tiles of 128 per instruction, fp32 PSUM):
    gate and candidate GEMMs for 4 o-tiles.
  - z = sigmoid(pg/S + bg), a = 1-z = sigmoid(-pg/S - bg) on ScalarE
    (descale folded into the activation scale; only Sigmoid is ever used so
    the ACT table is loaded exactly once).
  - bsc = (pc + S*bc) * z on VectorE (stt); S stays folded into the scan.
  - h' = tensor_tensor_scan(a, bsc) on VectorE, chained across chunks
    (h' = S*h -- the scan is linear in its additive input); DMA h' out.
The residual (out = h'/S + x) is applied on the host in exact fp32.
"""

import functools
import os
import numpy as np
import ml_dtypes

import concourse.bacc as bacc
import concourse.hw_specs as hw_specs
import concourse.tile as tile
from concourse import mybir
from concourse.bass_utils import run_bass_kernel_spmd

# The table-load pass assigns each activation the FIRST act_func_set that
# contains it; the preamble's Copy/Identity land in exp_and_others while
# Sigmoid lives in sigmoid_and_others, costing a second ~1.3us ACT_TABLE_LOAD
# that blocks the first sigmoid. Strip sigmoid_and_others' functions from
# every other set so a single table serves the whole kernel.
_orig_get_act_tables = hw_specs.get_activation_tables


@functools.cache
def _patched_get_act_tables(module_arch):
    d = dict(_orig_get_act_tables(module_arch))
    keep = "sigmoid_and_others"
    if keep in d:
        for name in d:
            if name != keep:
                d[name] = d[name] - d[keep]
    return d


hw_specs.get_activation_tables = _patched_get_act_tables
bacc.get_activation_tables = _patched_get_act_tables

B, T, H = 4, 4096, 1024
EPS = 1e-5
N_CORES = 8
OH = H // 2          # output channels per core
CHUNK = 512
N_CHUNKS = T // CHUNK
KT = H // 128        # k-tiles (contraction)
KP = KT // 2         # DoubleRow k-pairs per accumulation group
OT = OH // 128       # o-tiles per core
S = 256.0            # weight pre-scale so fp8 weights use e4m3 normal range

F32 = mybir.dt.float32
BF16 = mybir.dt.bfloat16
FP8 = mybir.dt.float8e4
AF = mybir.ActivationFunctionType
OP = mybir.AluOpType
DR = mybir.MatmulPerfMode.DoubleRow
F8 = ml_dtypes.float8_e4m3

_CACHE = {}


def _build():
    nc = bacc.Bacc("TRN2", target_bir_lowering=False, debug=False)

    # host-pre-tiled so every DMA is fully contiguous
    xn_d = nc.dram_tensor("xn", [N_CHUNKS, 128, KT, CHUNK], FP8, kind="ExternalInput").ap()
    wg_d = nc.dram_tensor("wg", [128, KT, OH], FP8, kind="ExternalInput").ap()
    wc_d = nc.dram_tensor("wc", [128, KT, OH], FP8, kind="ExternalInput").ap()
    bg_d = nc.dram_tensor("bg", [128, OT], F32, kind="ExternalInput").ap()
    bgn_d = nc.dram_tensor("bgn", [128, OT], F32, kind="ExternalInput").ap()
    bcs_d = nc.dram_tensor("bcs", [128, OT], F32, kind="ExternalInput").ap()
    out_d = nc.dram_tensor("outT", [N_CHUNKS, OT, 128, CHUNK], F32, kind="ExternalOutput").ap()

    with tile.TileContext(nc) as tc:
        with (
            tc.tile_pool(name="const", bufs=1) as cpool,
            tc.tile_pool(name="xin", bufs=3) as xpool,
            tc.tile_pool(name="work", bufs=2) as wpool,
            tc.tile_pool(name="hbuf", bufs=2) as hpool,
            tc.tile_pool(name="psA", bufs=4, space="PSUM") as psA,
            tc.tile_pool(name="psB", bufs=4, space="PSUM") as psB,
        ):
            # resident constants; weights ride the scalar queue so the first
            # x chunk (sync queue) isn't stuck behind them
            wg_sb = cpool.tile([128, KT, OH], FP8, tag="wg")
            nc.scalar.dma_start(wg_sb[:, 0:4, :], wg_d[:, 0:4, :])
            nc.scalar.dma_start(wg_sb[:, 4:KT, :], wg_d[:, 4:KT, :])
            wc_sb = cpool.tile([128, KT, OH], FP8, tag="wc")
            nc.scalar.dma_start(wc_sb[:, 0:4, :], wc_d[:, 0:4, :])
            nc.scalar.dma_start(wc_sb[:, 4:KT, :], wc_d[:, 4:KT, :])
            # bias DMAs are issued on the sync queue after xn0 (below): they
            # are tiny and must not queue behind 1 MiB of weights, or the
            # first stt/sigmoid chain starts ~5us late
            bg_sb = cpool.tile([128, OT], F32, tag="bg")
            bgn_sb = cpool.tile([128, OT], F32, tag="bgn")
            bcs_sb = cpool.tile([128, OT], F32, tag="bcs")


            h_prev = [None] * OT     # (tile, last column index)
            xc_t = [None] * N_CHUNKS

            def load_x(i, split=False):
                xc = xpool.tile([128, KT, CHUNK], FP8, tag="xc")
                if split:  # first chunk: let o=0's first k-pairs start earlier
                    half = KT // 2
                    nc.sync.dma_start(xc[:, :half, :], xn_d[i, :, :half, :])
                    nc.sync.dma_start(xc[:, half:, :], xn_d[i, :, half:, :])
                else:
                    nc.sync.dma_start(xc[:], xn_d[i])
                xc_t[i] = xc

            # chunk pairs (0,1),(2,3),(4,5) share one scan per o-tile on
            # VectorE ([128,1024] halves the per-instruction overhead); the
            # per-chunk stt writes into the pair tile's halves. Chunks 6,7
            # run single so the pipeline tail stays short.
            # chunks 0,1 also run single: the first pair's scan would
            # otherwise wait for chunk 1's stt, idling VectorE ~2us at start
            PAIRED = {2, 3, 4, 5}
            ap_t = [None] * OT   # per-o pair tiles carried even->odd chunk
            bp_t = [None] * OT

            load_x(0, split=True)
            nc.sync.dma_start(bg_sb[:], bg_d[:])
            nc.sync.dma_start(bgn_sb[:], bgn_d[:])
            nc.sync.dma_start(bcs_sb[:], bcs_d[:])
            load_x(1)
            for i in range(N_CHUNKS):
                if i + 1 < N_CHUNKS and i > 0:
                    load_x(i + 1)
                xc = xc_t[i]
                paired = i in PAIRED
                half = i % 2 if paired else 0
                for o in range(OT):
                    og = o * 128
                    pg = psA.tile([128, CHUNK], F32, tag="pg")
                    for j in range(KP):
                        nc.tensor.matmul(
                            pg[:], wg_sb[:, 2 * j : 2 * j + 2, og : og + 128],
                            xc[:, 2 * j : 2 * j + 2, :],
                            start=(j == 0), stop=(j == KP - 1), perf_mode=DR,
                        )
                    pc = psB.tile([128, CHUNK], F32, tag="pc")
                    for j in range(KP):
                        nc.tensor.matmul(
                            pc[:], wc_sb[:, 2 * j : 2 * j + 2, og : og + 128],
                            xc[:, 2 * j : 2 * j + 2, :],
                            start=(j == 0), stop=(j == KP - 1), perf_mode=DR,
                        )

                    if paired and half == 0:
                        ap_t[o] = wpool.tile([128, 2 * CHUNK], F32, tag=f"a{o}", name="apair")
                        bp_t[o] = wpool.tile([128, 2 * CHUNK], F32, tag=f"b{o}", name="bpair")
                    if paired:
                        a = ap_t[o][:, half * CHUNK : (half + 1) * CHUNK]
                        bsc = bp_t[o][:, half * CHUNK : (half + 1) * CHUNK]
                    else:
                        a_t = wpool.tile([128, CHUNK], F32, tag=f"a{o}", name="a_t")
                        b_t = wpool.tile([128, CHUNK], F32, tag=f"b{o}", name="b_t")
                        a = a_t[:]
                        bsc = b_t[:]
                    z = wpool.tile([128, CHUNK], F32, tag="z", bufs=4)
                    nc.scalar.activation(
                        z[:], pg[:], AF.Sigmoid, bias=bg_sb[:, o : o + 1], scale=1.0 / S
                    )
                    nc.scalar.activation(
                        a, pg[:], AF.Sigmoid, bias=bgn_sb[:, o : o + 1], scale=-1.0 / S
                    )
                    nc.vector.scalar_tensor_tensor(
                        bsc, pc[:], bcs_sb[:, o : o + 1], z[:], OP.add, OP.mult
                    )

                    if paired and half == 0:
                        continue  # scan fires on the odd half over [128,1024]
                    if paired:
                        af, bf_, W = ap_t[o][:], bp_t[o][:], 2 * CHUNK
                    else:
                        af, bf_, W = a, bsc, CHUNK
                    h = hpool.tile([128, W], F32, tag=f"h{o}")
                    if h_prev[o] is None:
                        init = 0.0
                    else:
                        pt, pcol = h_prev[o]
                        init = pt[:, pcol : pcol + 1]
                    nc.vector.tensor_tensor_scan(h[:], af, bf_, init, OP.mult, OP.add)
                    h_prev[o] = (h, W - 1)
                    if paired:
                        nc.sync.dma_start(out_d[i - 1, o], h[:, 0:CHUNK])
                        nc.sync.dma_start(out_d[i, o], h[:, CHUNK : 2 * CHUNK])
                    else:
                        nc.sync.dma_start(out_d[i, o], h[:])

    nc.compile()
    return nc


def _prep_weights(gamma, beta, Wg, bg, Wc, bc, ohalf):
    """Host-side weight folding for one output half.

    h-rows of the weights (and of xn/xr) are rolled so this half's own output
    channels come first: the device residual then always reads x rows at
    k-tiles 0..OT-1 with one shared program across cores.
    """
    o0 = ohalf * OH
    perm = np.roll(np.arange(H), -o0)
    Wg_h = Wg[o0 : o0 + OH]          # [OH, H]
    Wc_h = Wc[o0 : o0 + OH]
    # lhsT layout [h, o], gamma folded into rows (h), rows permuted like xn
    wg_eff = ((Wg_h * gamma[None, :]).T)[perm]   # [H, OH]
    wc_eff = ((Wc_h * gamma[None, :]).T)[perm]
    bg_eff = (bg[o0 : o0 + OH] + Wg_h @ beta).astype(np.float32)
    bc_eff = (bc[o0 : o0 + OH] + Wc_h @ beta).astype(np.float32)

    def q8(w):  # [H, OH] -> fp8 tiles [128, KT, OH]
        w8 = np.clip(S * w, -240, 240).astype(F8)
        return np.ascontiguousarray(w8.reshape(KT, 128, OH).transpose(1, 0, 2))

    return {
        "wg": q8(wg_eff),
        "wc": q8(wc_eff),
        "bg": np.ascontiguousarray(bg_eff.reshape(OT, 128).T),
        "bgn": np.ascontiguousarray(-bg_eff.reshape(OT, 128).T),
        "bcs": np.ascontiguousarray(S * bc_eff.reshape(OT, 128).T),
    }


def kernel(x, gamma, beta, Wg, bg, Wc, bc):
    x = np.asarray(x, dtype=np.float32)
    gamma = np.asarray(gamma, dtype=np.float32)
    beta = np.asarray(beta, dtype=np.float32)
    Wg = np.asarray(Wg, dtype=np.float32)
    bg = np.asarray(bg, dtype=np.float32)
    Wc = np.asarray(Wc, dtype=np.float32)
    bc = np.asarray(bc, dtype=np.float32)

    if "nc" not in _CACHE:
        _CACHE["nc"] = _build()
    nc = _CACHE["nc"]

    # exact LN on host; gamma/beta fold into the weights/biases
    mu = x.mean(-1, keepdims=True)
    var = ((x - mu) ** 2).mean(-1, keepdims=True)
    normed = (x - mu) / np.sqrt(var + EPS)
    xn8 = np.clip(normed, -240, 240).astype(F8)

    halves = [_prep_weights(gamma, beta, Wg, bg, Wc, bc, p) for p in range(2)]

    in_maps = []
    for c in range(N_CORES):
        b, p = divmod(c, 2)
        m = dict(halves[p])
        # roll h-rows to match the weight-row permutation for this half
        xnT = xn8[b].T if p == 0 else np.roll(xn8[b].T, -OH, axis=0)   # [H, T]
        m["xn"] = np.ascontiguousarray(
            xnT.reshape(KT, 128, N_CHUNKS, CHUNK).transpose(2, 1, 0, 3)
        )
        in_maps.append(m)

    trace = bool(int(os.environ.get("MINGRU_TRACE", "0")))
    kwargs = {}
    if trace:
        tmpdir = os.environ.get("MINGRU_TRACE_DIR") or None
        kwargs = dict(trace=True, tmpdir=tmpdir)
    res = run_bass_kernel_spmd(nc, in_maps, core_ids=list(range(N_CORES)), **kwargs)
    if trace:
        _CACHE["last_results"] = res

    out = np.empty((B, T, H), dtype=np.float32)
    for c in range(N_CORES):
        b, p = divmod(c, 2)
        # [chunks, OT, 128, CHUNK] -> [OH, T] -> [T, OH]; h'/S + x residual
        oT = res.results[c]["outT"].transpose(1, 2, 0, 3).reshape(OH, T)
        sl = slice(p * OH, (p + 1) * OH)
        out[b, :, sl] = oT.T * (1.0 / S) + x[b, :, sl]
    return out


# revision 51
# speedup vs baseline: 1.0235x; 1.0131x over previous
"""MinGRU layer (LN -> gate/candidate Linear -> minGRU scan -> residual) on 8 trn2 cores.

Problem (hardcoded): x [B=4, T=4096, H=1024] fp32, weights Wg/Wc [1024,1024],
biases bg/bc [1024], LN gamma/beta [1024].

Sharding: core c = (batch b = c//2, output-half p = c%2). Each core computes
z/c for its 512 output channels over the full sequence; the minGRU recurrence
is elementwise over (b, h) so output-channel sharding needs no collectives.

Strategy: all LayerNorm work (mu/var/rstd + normalize) happens on the HOST in
exact fp32; the device receives the already-normalized x quantized to fp8e4
plus fp8e4 weights pre-scaled by S=256 (so |S*W| stays in e4m3's normal
range). Device work per 512-token chunk is then just:
  - 32 DoubleRow fp8 matmuls (2 k-tiles of 128 per instruction, fp32 PSUM):
    gate and candidate GEMMs for 4 o-tiles.
  - z = sigmoid(pg/S + bg), a = 1-z = sigmoid(-pg/S - bg) on ScalarE
    (descale folded into the activation scale; only Sigmoid is ever used so
    the ACT table is loaded exactly once).
  - bsc = (pc + S*bc) * z on VectorE (stt); S stays folded into the scan.
  - h' = tensor_tensor_scan(a, bsc) on VectorE, chained across chunks
    (h' = S*h -- the scan is linear in its additive input); DMA h' out.
The residual (out = h'/S + x) is applied on the host in exact fp32.
"""

import functools
import os
import numpy as np
import ml_dtypes

import concourse.bacc as bacc
import concourse.hw_specs as hw_specs
import concourse.tile as tile
from concourse import mybir
from concourse.bass_utils import run_bass_kernel_spmd

# The table-load pass assigns each activation the FIRST act_func_set that
# contains it; the preamble's Copy/Identity land in exp_and_others while
# Sigmoid lives in sigmoid_and_others, costing a second ~1.3us ACT_TABLE_LOAD
# that blocks the first sigmoid. Strip sigmoid_and_others' functions from
# every other set so a single table serves the whole kernel.
_orig_get_act_tables = hw_specs.get_activation_tables


@functools.cache
def _patched_get_act_tables(module_arch):
    d = dict(_orig_get_act_tables(module_arch))
    keep = "sigmoid_and_others"
    if keep in d:
        for name in d:
            if name != keep:
                d[name] = d[name] - d[keep]
    return d


hw_specs.get_activation_tables = _patched_get_act_tables
bacc.get_activation_tables = _patched_get_act_tables

B, T, H = 4, 4096, 1024
EPS = 1e-5
N_CORES = 8
OH = H // 2          # output channels per core
CHUNK = 512
N_CHUNKS = T // CHUNK
KT = H // 128        # k-tiles (contraction)
KP = KT // 2         # DoubleRow k-pairs per accumulation group
OT = OH // 128       # o-tiles per core
S = 256.0            # weight pre-scale so fp8 weights use e4m3 normal range

F32 = mybir.dt.float32
BF16 = mybir.dt.bfloat16
FP8 = mybir.dt.float8e4
AF = mybir.ActivationFunctionType
OP = mybir.AluOpType
DR = mybir.MatmulPerfMode.DoubleRow
F8 = ml_dtypes.float8_e4m3

_CACHE = {}


def _build():
    nc = bacc.Bacc("TRN2", target_bir_lowering=False, debug=False)

    # host-pre-tiled so every DMA is fully contiguous
    xn_d = nc.dram_tensor("xn", [N_CHUNKS, 128, KT, CHUNK], FP8, kind="ExternalInput").ap()
    wg_d = nc.dram_tensor("wg", [128, KT, OH], FP8, kind="ExternalInput").ap()
    wc_d = nc.dram_tensor("wc", [128, KT, OH], FP8, kind="ExternalInput").ap()
    bg_d = nc.dram_tensor("bg", [128, OT], F32, kind="ExternalInput").ap()
    bgn_d = nc.dram_tensor("bgn", [128, OT], F32, kind="ExternalInput").ap()
    bcs_d = nc.dram_tensor("bcs", [128, OT], F32, kind="ExternalInput").ap()
    out_d = nc.dram_tensor("outT", [N_CHUNKS, OT, 128, CHUNK], F32, kind="ExternalOutput").ap()

    with tile.TileContext(nc) as tc:
        with (
            tc.tile_pool(name="const", bufs=1) as cpool,
            tc.tile_pool(name="xin", bufs=3) as xpool,
            tc.tile_pool(name="work", bufs=2) as wpool,
            tc.tile_pool(name="hbuf", bufs=2) as hpool,
            tc.tile_pool(name="psA", bufs=4, space="PSUM") as psA,
            tc.tile_pool(name="psB", bufs=4, space="PSUM") as psB,
        ):
            # resident constants; weights ride the scalar queue so the first
            # x chunk (sync queue) isn't stuck behind them
            wg_sb = cpool.tile([128, KT, OH], FP8, tag="wg")
            nc.scalar.dma_start(wg_sb[:, 0:4, :], wg_d[:, 0:4, :])
            nc.scalar.dma_start(wg_sb[:, 4:KT, :], wg_d[:, 4:KT, :])
            wc_sb = cpool.tile([128, KT, OH], FP8, tag="wc")
            nc.scalar.dma_start(wc_sb[:, 0:4, :], wc_d[:, 0:4, :])
            nc.scalar.dma_start(wc_sb[:, 4:KT, :], wc_d[:, 4:KT, :])
            # bias DMAs are issued on the sync queue after xn0 (below): they
            # are tiny and must not queue behind 1 MiB of weights, or the
            # first stt/sigmoid chain starts ~5us late
            bg_sb = cpool.tile([128, OT], F32, tag="bg")
            bgn_sb = cpool.tile([128, OT], F32, tag="bgn")
            bcs_sb = cpool.tile([128, OT], F32, tag="bcs")


            h_prev = [None] * OT     # (tile, last column index)
            xc_t = [None] * N_CHUNKS

            def load_x(i, split=False):
                xc = xpool.tile([128, KT, CHUNK], FP8, tag="xc")
                if split:  # first chunk: let o=0's first k-pairs start earlier
                    half = KT // 2
                    nc.sync.dma_start(xc[:, :half, :], xn_d[i, :, :half, :])
                    nc.sync.dma_start(xc[:, half:, :], xn_d[i, :, half:, :])
                else:
                    nc.sync.dma_start(xc[:], xn_d[i])
                xc_t[i] = xc

            # chunk pairs (0,1),(2,3),(4,5) share one scan per o-tile on
            # VectorE ([128,1024] halves the per-instruction overhead); the
            # per-chunk stt writes into the pair tile's halves. Chunks 6,7
            # run single so the pipeline tail stays short.
            # chunks 0,1 also run single: the first pair's scan would
            # otherwise wait for chunk 1's stt, idling VectorE ~2us at start
            PAIRED = {2, 3, 4, 5}
            ap_t = [None] * OT   # per-o pair tiles carried even->odd chunk
            bp_t = [None] * OT

            load_x(0, split=True)
            nc.sync.dma_start(bg_sb[:], bg_d[:])
            nc.sync.dma_start(bgn_sb[:], bgn_d[:])
            nc.sync.dma_start(bcs_sb[:], bcs_d[:])
            load_x(1)
            for i in range(N_CHUNKS):
                if i + 1 < N_CHUNKS and i > 0:
                    load_x(i + 1)
                xc = xc_t[i]
                paired = i in PAIRED
                half = i % 2 if paired else 0
                for o in range(OT):
                    og = o * 128
                    pg = psA.tile([128, CHUNK], F32, tag="pg")
                    for j in range(KP):
                        nc.tensor.matmul(
                            pg[:], wg_sb[:, 2 * j : 2 * j + 2, og : og + 128],
                            xc[:, 2 * j : 2 * j + 2, :],
                            start=(j == 0), stop=(j == KP - 1), perf_mode=DR,
                        )
                    pc = psB.tile([128, CHUNK], F32, tag="pc")
                    for j in range(KP):
                        nc.tensor.matmul(
                            pc[:], wc_sb[:, 2 * j : 2 * j + 2, og : og + 128],
                            xc[:, 2 * j : 2 * j + 2, :],
                            start=(j == 0), stop=(j == KP - 1), perf_mode=DR,
                        )

                    if paired and half == 0:
                        ap_t[o] = wpool.tile([128, 2 * CHUNK], F32, tag=f"a{o}", name="apair")
                        bp_t[o] = wpool.tile([128, 2 * CHUNK], BF16, tag=f"b{o}", name="bpair")
                    if paired:
                        a = ap_t[o][:, half * CHUNK : (half + 1) * CHUNK]
                        bsc = bp_t[o][:, half * CHUNK : (half + 1) * CHUNK]
                    else:
                        a_t = wpool.tile([128, CHUNK], F32, tag=f"a{o}", name="a_t")
                        b_t = wpool.tile([128, CHUNK], BF16, tag=f"b{o}", name="b_t")
                        a = a_t[:]
                        bsc = b_t[:]
                    z = wpool.tile([128, CHUNK], BF16, tag="z", bufs=4)
                    nc.scalar.activation(
                        z[:], pg[:], AF.Sigmoid, bias=bg_sb[:, o : o + 1], scale=1.0 / S
                    )
                    nc.scalar.activation(
                        a, pg[:], AF.Sigmoid, bias=bgn_sb[:, o : o + 1], scale=-1.0 / S
                    )
                    if o % 2 == 0:
                        # rebalance: ScalarE (under-used) applies the bias,
                        # VectorE does only a 2x-rate bf16 multiply -- cuts
                        # the saturated VectorE's per-o cost ~2x for half
                        # the o-tiles
                        t = wpool.tile([128, CHUNK], BF16, tag="t", bufs=4)
                        nc.scalar.activation(
                            t[:], pc[:], AF.Identity, bias=bcs_sb[:, o : o + 1]
                        )
                        nc.vector.tensor_mul(bsc, t[:], z[:])
                    else:
                        nc.vector.scalar_tensor_tensor(
                            bsc, pc[:], bcs_sb[:, o : o + 1], z[:], OP.add, OP.mult
                        )

                    if paired and half == 0:
                        continue  # scan fires on the odd half over [128,1024]
                    if paired:
                        af, bf_, W = ap_t[o][:], bp_t[o][:], 2 * CHUNK
                    else:
                        af, bf_, W = a, bsc, CHUNK
                    h = hpool.tile([128, W], F32, tag=f"h{o}")
                    if h_prev[o] is None:
                        init = 0.0
                    else:
                        pt, pcol = h_prev[o]
                        init = pt[:, pcol : pcol + 1]
                    nc.vector.tensor_tensor_scan(h[:], af, bf_, init, OP.mult, OP.add)
                    h_prev[o] = (h, W - 1)
                    if paired:
                        nc.sync.dma_start(out_d[i - 1, o], h[:, 0:CHUNK])
                        nc.sync.dma_start(out_d[i, o], h[:, CHUNK : 2 * CHUNK])
                    else:
                        nc.sync.dma_start(out_d[i, o], h[:])

    nc.compile()
    return nc


def _prep_weights(gamma, beta, Wg, bg, Wc, bc, ohalf):
    """Host-side weight folding for one output half.

    h-rows of the weights (and of xn/xr) are rolled so this half's own output
    channels come first: the device residual then always reads x rows at
    k-tiles 0..OT-1 with one shared program across cores.
    """
    o0 = ohalf * OH
    perm = np.roll(np.arange(H), -o0)
    Wg_h = Wg[o0 : o0 + OH]          # [OH, H]
    Wc_h = Wc[o0 : o0 + OH]
    # lhsT layout [h, o], gamma folded into rows (h), rows permuted like xn
    wg_eff = ((Wg_h * gamma[None, :]).T)[perm]   # [H, OH]
    wc_eff = ((Wc_h * gamma[None, :]).T)[perm]
    bg_eff = (bg[o0 : o0 + OH] + Wg_h @ beta).astype(np.float32)
    bc_eff = (bc[o0 : o0 + OH] + Wc_h @ beta).astype(np.float32)

    def q8(w):  # [H, OH] -> fp8 tiles [128, KT, OH]
        w8 = np.clip(S * w, -240, 240).astype(F8)
        return np.ascontiguousarray(w8.reshape(KT, 128, OH).transpose(1, 0, 2))

    return {
        "wg": q8(wg_eff),
        "wc": q8(wc_eff),
        "bg": np.ascontiguousarray(bg_eff.reshape(OT, 128).T),
        "bgn": np.ascontiguousarray(-bg_eff.reshape(OT, 128).T),
        "bcs": np.ascontiguousarray(S * bc_eff.reshape(OT, 128).T),
    }


def kernel(x, gamma, beta, Wg, bg, Wc, bc):
    x = np.asarray(x, dtype=np.float32)
    gamma = np.asarray(gamma, dtype=np.float32)
    beta = np.asarray(beta, dtype=np.float32)
    Wg = np.asarray(Wg, dtype=np.float32)
    bg = np.asarray(bg, dtype=np.float32)
    Wc = np.asarray(Wc, dtype=np.float32)
    bc = np.asarray(bc, dtype=np.float32)

    if "nc" not in _CACHE:
        _CACHE["nc"] = _build()
    nc = _CACHE["nc"]

    # exact LN on host; gamma/beta fold into the weights/biases
    mu = x.mean(-1, keepdims=True)
    var = ((x - mu) ** 2).mean(-1, keepdims=True)
    normed = (x - mu) / np.sqrt(var + EPS)
    xn8 = np.clip(normed, -240, 240).astype(F8)

    halves = [_prep_weights(gamma, beta, Wg, bg, Wc, bc, p) for p in range(2)]

    in_maps = []
    for c in range(N_CORES):
        b, p = divmod(c, 2)
        m = dict(halves[p])
        # roll h-rows to match the weight-row permutation for this half
        xnT = xn8[b].T if p == 0 else np.roll(xn8[b].T, -OH, axis=0)   # [H, T]
        m["xn"] = np.ascontiguousarray(
            xnT.reshape(KT, 128, N_CHUNKS, CHUNK).transpose(2, 1, 0, 3)
        )
        in_maps.append(m)

    trace = bool(int(os.environ.get("MINGRU_TRACE", "0")))
    kwargs = {}
    if trace:
        tmpdir = os.environ.get("MINGRU_TRACE_DIR") or None
        kwargs = dict(trace=True, tmpdir=tmpdir)
    res = run_bass_kernel_spmd(nc, in_maps, core_ids=list(range(N_CORES)), **kwargs)
    if trace:
        _CACHE["last_results"] = res

    out = np.empty((B, T, H), dtype=np.float32)
    for c in range(N_CORES):
        b, p = divmod(c, 2)
        # [chunks, OT, 128, CHUNK] -> [OH, T] -> [T, OH]; h'/S + x residual
        oT = res.results[c]["outT"].transpose(1, 2, 0, 3).reshape(OH, T)
        sl = slice(p * OH, (p + 1) * OH)
        out[b, :, sl] = oT.T * (1.0 / S) + x[b, :, sl]
    return out
